# revision 41
# baseline (speedup 1.0000x reference)
"""Trainium2 Bass kernel for nn_BailingMoEAttention (B=2, S=2048, HID=2048,
NH=16, NKV=4, HD=128) on 8 NeuronCores.

Sharding: core c -> (batch b = c//4, kv-group g = c%4). Each core computes the
4 query heads sharing kv head g for batch b against its Wq/Wk/Wv column slices
and Wc row slice, producing a partial [S, HID] output; the host sums the 4
group partials per batch.

v2 design (per core):
  - Q projection runs as fp8e4 DoubleRow matmuls (HW-measured 2.06x over
    bf16): xt and Wq are host-scaled into fp8 range; rmsnorm cancels the
    scale (eps is pre-scaled to match). Q-side fp8 noise only perturbs
    softmax logits (~1e-2 relative), unlike the V/out-proj paths which
    propagate element-wise and would blow the 2e-2 budget - those stay bf16.
  - K/V projection in bf16 from an s-block-streamed bf16 x^T copy.
  - attention bf16: scores ST[k,q] = K-chunk^T @ Q^T, exp on ACT from PSUM;
    causal handling is exact at 128 granularity: fully-masked q sub-blocks
    are skipped in the scores matmul, exp, and AV accumulation; the diagonal
    [128,128] triangle is zeroed on Pool (affine_select) after exp.
  - softmax denominators ride as a ones-column in V; normalize is
    reciprocal+per-partition scale on DVE.
  - engine budget: ACT = exp + squares(+accum) + sqrt + qt drain; Pool =
    k-path (combk/kn/rope-k), rope-q heads 2-3, kt/v drains, diagonal
    affine_selects, half the out-proj drains; DVE = recip/comb/qn, rope-q
    heads 0-1, o_sb normalize, O^T copies, other half of out-proj drains.
  - all inputs arrive host-prearranged as [128, chunk, cols] so each tensor
    loads with a few large contiguous dma_starts (descriptor-efficient).

reps>1 compiles a NEFF running the whole body reps times back-to-back; used
by test.py to measure steady-state body time as the reps-delta.
"""
import os
import sys
sys.path.insert(0, "/opt/trn_rl_repo")

from contextlib import ExitStack

import numpy as np

import concourse.bass as bass
import concourse.tile as tile
from concourse import bacc, mybir
from concourse.masks import make_identity

F32 = mybir.dt.float32
BF16 = mybir.dt.bfloat16
F8 = mybir.dt.float8e4

B, S, HID = 2, 2048, 2048
NH, NKV, HD = 16, 4, 128
NHL = NH // NKV          # query heads per kv group (= per core)
DQ = NHL * HD
EPS = 1e-6
THETA = 10000.0
N_CORES = 8

S_X = 64.0               # fp8 scale on hidden_states^T (q path)
S_W = 32.0               # fp8 scale on Wq
S_QK = S_X * S_W         # q_ps comes out scaled by this; rmsnorm cancels it


def _build(reps=1, phases="123"):
    n_st = S // 128
    n_hc = HID // 128
    n_qb = S // 512
    n_sg = S // 256
    half = HD // 2
    DR = mybir.MatmulPerfMode.DoubleRow

    nc = bacc.Bacc("TRN2", target_bir_lowering=False, debug=False, num_devices=1)
    # q path: chunk-major fp8 x^T, resident
    xq_d = nc.dram_tensor("xq", [128, n_hc, S], F8, kind="ExternalInput").ap()
    # kv path: s-block-major bf16 x^T, streamed
    xb_d = nc.dram_tensor("xb", [128, n_sg, n_hc, 256], BF16,
                          kind="ExternalInput").ap()
    wq_d = nc.dram_tensor("wq", [128, n_hc, DQ], F8, kind="ExternalInput").ap()
    wk_d = nc.dram_tensor("wk", [128, n_hc, HD], F8, kind="ExternalInput").ap()
    wv_d = nc.dram_tensor("wv", [128, n_hc, HD], BF16,
                          kind="ExternalInput").ap()
    wc_d = nc.dram_tensor("wc", [128, NHL, HID], BF16, kind="ExternalInput").ap()
    qs_d = nc.dram_tensor("qs", [DQ], F32, kind="ExternalInput").ap()
    ks_d = nc.dram_tensor("ks", [HD], F32, kind="ExternalInput").ap()
    cos_d = nc.dram_tensor("cos", [S, half], BF16, kind="ExternalInput").ap()
    sin_d = nc.dram_tensor("sin", [S, half], BF16, kind="ExternalInput").ap()
    out_d = nc.dram_tensor("out", [S, HID], BF16, kind="ExternalOutput").ap()

    with tile.TileContext(nc) as tc, ExitStack() as ctx:
        const_p = ctx.enter_context(tc.tile_pool(name="const", bufs=1))
        big_p = ctx.enter_context(tc.tile_pool(name="big", bufs=1))

        ident = const_p.tile([128, 128], BF16)
        make_identity(nc, ident)
        # lower-triangle (q >= k) mask for diagonal score blocks
        tri = const_p.tile([128, 128], BF16)
        nc.gpsimd.memset(tri, 1.0)
        nc.gpsimd.affine_select(
            out=tri, in_=tri, compare_op=mybir.AluOpType.is_ge, fill=0.0,
            base=0, pattern=[[1, 128]], channel_multiplier=-1)
        eps_t = const_p.tile([128, 1], F32)
        # rmsnorm runs on S_QK-scaled q (and k stats are scale-matched via
        # the Square(scale=S_QK) trick), so eps scales by S_QK^2
        nc.vector.memset(eps_t, EPS * S_QK * S_QK)
        qs_b = const_p.tile([128, DQ], F32)
        nc.scalar.dma_start(out=qs_b, in_=bass.AP(tensor=qs_d.tensor, offset=0,
                                                  ap=[[0, 128]] + qs_d.ap))
        ks_b = const_p.tile([128, HD], F32)
        nc.scalar.dma_start(out=ks_b, in_=bass.AP(tensor=ks_d.tensor, offset=0,
                                                  ap=[[0, 128]] + ks_d.ap))
        cs_sb = const_p.tile([128, n_st, half], BF16)
        sn_sb = const_p.tile([128, n_st, half], BF16)
        nc.scalar.dma_start(out=cs_sb, in_=cos_d.rearrange("(t p) h -> p t h",
                                                           p=128))
        nc.scalar.dma_start(out=sn_sb, in_=sin_d.rearrange("(t p) h -> p t h",
                                                           p=128))

        xq_f = big_p.tile([128, n_hc, S], F8)
        wq_sb = big_p.tile([128, n_hc, DQ], F8)
        wk_sb = big_p.tile([128, n_hc, HD], F8)
        wv_sb = big_p.tile([128, n_hc, HD], BF16)
        wc_sb = big_p.tile([128, NHL, HID], BF16)
        qt_sb = big_p.tile([128, n_st, NHL, 128], BF16)  # [d,(st,head,qq)]
        kt_sb = big_p.tile([128, n_st, 128], BF16)       # [d,(chunk,kk)]
        v_sb = big_p.tile([128, n_st, HD + 1], BF16)     # [kk,(chunk, d|ones)]
        nc.vector.memset(v_sb[:, :, HD:HD + 1], 1.0)

        for _rep in range(reps):
            # weight/x loads: few large contiguous DMAs, ordered so the first
            # QKV matmuls unblock earliest
            # small first pieces so the first matmuls unblock within ~3us;
            # late pieces ride the otherwise-idle ACT queue, wc on Pool
            nc.sync.dma_start(out=wq_sb[:, 0:4, :], in_=wq_d[:, 0:4, :])
            nc.gpsimd.dma_start(out=xq_f[:, 0:4, :], in_=xq_d[:, 0:4, :])
            nc.sync.dma_start(out=wk_sb, in_=wk_d)
            nc.sync.dma_start(out=wv_sb[:, 0:8, :], in_=wv_d[:, 0:8, :])
            nc.sync.dma_start(out=wq_sb[:, 4:8, :], in_=wq_d[:, 4:8, :])
            nc.gpsimd.dma_start(out=xq_f[:, 4:16, :], in_=xq_d[:, 4:16, :])
            nc.scalar.dma_start(out=wq_sb[:, 8:16, :], in_=wq_d[:, 8:16, :])
            nc.scalar.dma_start(out=wv_sb[:, 8:16, :], in_=wv_d[:, 8:16, :])
            nc.gpsimd.dma_start(out=wc_sb[:, 0:2, :], in_=wc_d[:, 0:2, :])
            nc.gpsimd.dma_start(out=wc_sb[:, 2:4, :], in_=wc_d[:, 2:4, :])

            # ============ Phase 1: QKV + norm + rope + transposes ============
            with tc.tile_pool(name="p1xb", bufs=3) as xb_p, \
                 tc.tile_pool(name="p1q", bufs=3, space="PSUM") as qps_p, \
                 tc.tile_pool(name="p1kv", bufs=3, space="PSUM") as kvps_p, \
                 tc.tile_pool(name="p1tmp", bufs=4) as tmp_p:
                def p1_stats(st, q_ps, kv_ps):
                    # rms stats: squares+accumulate on ACT, per head; k stats
                    # are computed on S_QK*k so one eps/bias serves both
                    junk = tmp_p.tile([128, 128], BF16, tag="junk")
                    ssq = tmp_p.tile([128, NHL + 1], F32, tag="ssq")
                    for h in range(NHL):
                        nc.scalar.activation(
                            junk, q_ps[:, h * HD:(h + 1) * HD],
                            mybir.ActivationFunctionType.Square,
                            accum_out=ssq[:, h:h + 1])
                    nc.scalar.activation(
                        junk, kv_ps[:, 0:HD],
                        mybir.ActivationFunctionType.Square,
                        accum_out=ssq[:, NHL:NHL + 1])
                    rstd = tmp_p.tile([128, NHL + 1], F32, tag="rstd")
                    nc.scalar.activation(rstd, ssq,
                                         mybir.ActivationFunctionType.Sqrt,
                                         bias=eps_t, scale=1.0 / HD)
                    nc.vector.reciprocal(rstd, rstd)
                    # q+k normalize into ONE tile so rope runs as 6 wide Pool
                    # ops over 5 "heads" (4 q heads + k) — GPSIMD ops carry a
                    # large unmodeled fixed cost on HW, so minimize their count
                    qkn = tmp_p.tile([128, NHL + 1, HD], BF16, tag="qkn")
                    comb = tmp_p.tile([128, DQ], F32, tag="comb")
                    rstd_b = bass.AP(
                        tensor=rstd.tensor, offset=rstd.offset,
                        ap=[rstd.ap[0], [rstd.ap[-1][0], NHL], [0, HD]])
                    nc.vector.tensor_mul(
                        comb.rearrange("p (h d) -> p h d", d=HD),
                        qs_b.rearrange("p (h d) -> p h d", d=HD), rstd_b)
                    nc.vector.tensor_mul(
                        qkn[:, 0:NHL, :], q_ps.rearrange("p (h d) -> p h d",
                                                         d=HD), comb.rearrange(
                            "p (h d) -> p h d", d=HD))
                    # k: combk = ks * rstd[k] (ks pre-scaled by S_QK on host)
                    combk = tmp_p.tile([128, HD], F32, tag="combk")
                    nc.gpsimd.tensor_scalar_mul(combk, ks_b,
                                                rstd[:, NHL:NHL + 1])
                    nc.vector.tensor_mul(qkn[:, NHL, :], kv_ps[:, 0:HD], combk)
                    nc.vector.tensor_copy(v_sb[:, st, 0:HD], kv_ps[:, HD:2 * HD])
                    return qkn

                def p1_finish(st, qkn):
                    # rope (bf16): 6 Pool ops over all 5 heads at once
                    qkr = tmp_p.tile([128, NHL + 1, HD], BF16, tag="qkr")
                    s3 = qkn.rearrange("p h (two d) -> p h two d", two=2)
                    d3 = qkr.rearrange("p h (two d) -> p h two d", two=2)
                    x1, x2 = s3[:, :, 0, :], s3[:, :, 1, :]
                    o1, o2 = d3[:, :, 0, :], d3[:, :, 1, :]
                    cs_t = cs_sb[:, st, :]
                    sn_t = sn_sb[:, st, :]
                    cb = bass.AP(tensor=cs_t.tensor, offset=cs_t.offset,
                                 ap=[cs_t.ap[0], [0, NHL + 1]] + cs_t.ap[1:])
                    sb = bass.AP(tensor=sn_t.tensor, offset=sn_t.offset,
                                 ap=[sn_t.ap[0], [0, NHL + 1]] + sn_t.ap[1:])
                    t1 = tmp_p.tile([128, NHL + 1, half], BF16, tag="ropet1")
                    t2 = tmp_p.tile([128, NHL + 1, half], BF16, tag="ropet2")
                    nc.gpsimd.tensor_mul(t1, x1, cb)
                    nc.gpsimd.tensor_mul(t2, x2, sb)
                    nc.gpsimd.tensor_sub(o1, t1, t2)
                    nc.gpsimd.tensor_mul(t1, x2, cb)
                    nc.gpsimd.tensor_mul(t2, x1, sb)
                    nc.gpsimd.tensor_add(o2, t1, t2)

                    # transpose q/k via the DMA crossbar: no PE/drain cost,
                    # only queue-issue time (SP for q heads, ACT for k)
                    for h in range(NHL):
                        nc.sync.dma_start_transpose(
                            out=qt_sb[:, st, h, :], in_=qkr[:, h, :])
                    nc.scalar.dma_start_transpose(
                        out=kt_sb[:, st, :], in_=qkr[:, NHL, :])

                # software pipeline, 2 tail stages deep: for iteration st we
                # emit MMs(st), stats(st-1), finish(st-2) so every engine's
                # strict FIFO always holds ready work (a single monolithic
                # tail head-of-line-blocks each queue on the slow chain)
                stats_q, fin_q = [], []
                for sg in range(n_sg):
                    xb_t = xb_p.tile([128, n_hc, 256], BF16, tag="xb")
                    nc.sync.dma_start(out=xb_t, in_=xb_d[:, sg, :, :])
                    for t in range(2):
                        st = sg * 2 + t
                        q_ps = qps_p.tile([128, DQ], F32, tag="qps")
                        kv_ps = kvps_p.tile([128, 2 * HD], F32, tag="kvps")
                        for c8 in range(n_hc // 2):
                            lhsT = xq_f[:, 2 * c8:2 * c8 + 2,
                                        st * 128:(st + 1) * 128]
                            nc.tensor.matmul(
                                q_ps[:], lhsT, wq_sb[:, 2 * c8:2 * c8 + 2, :],
                                start=(c8 == 0), stop=(c8 == n_hc // 2 - 1),
                                perf_mode=DR)
                            nc.tensor.matmul(
                                kv_ps[:, 0:HD], lhsT,
                                wk_sb[:, 2 * c8:2 * c8 + 2, :],
                                start=(c8 == 0), stop=(c8 == n_hc // 2 - 1),
                                perf_mode=DR)
                        for c in range(n_hc):
                            nc.tensor.matmul(
                                kv_ps[:, HD:2 * HD],
                                xb_t[:, c, t * 128:(t + 1) * 128],
                                wv_sb[:, c, :],
                                start=(c == 0), stop=(c == n_hc - 1))
                        stats_q.append((st, q_ps, kv_ps))
                        if len(stats_q) > 1:
                            s_st, s_q, s_kv = stats_q.pop(0)
                            fin_q.append((s_st, p1_stats(s_st, s_q, s_kv)))
                        if len(fin_q) > 1:
                            p1_finish(*fin_q.pop(0))
                for s_st, s_q, s_kv in stats_q:
                    fin_q.append((s_st, p1_stats(s_st, s_q, s_kv)))
                for args in fin_q:
                    p1_finish(*args)

            if phases == "1":
                nc.sync.dma_start(
                    out=out_d[0:128, :],
                    in_=qt_sb.rearrange("p a b c -> p (a b c)")[:, 0:HID])
                continue
            # full normalized O^T, [d, (head, q)] bf16 (2 MB)
            ot_full = big_p.tile([128, NHL, S], BF16)

            # ===== Phase 2+3: attention with interleaved out-proj rows =====
            with tc.tile_pool(name="a_st", bufs=2, space="PSUM") as st_ps_p, \
                 tc.tile_pool(name="a_o", bufs=2, space="PSUM") as o_ps_p, \
                 tc.tile_pool(name="a_op", bufs=3, space="PSUM") as op_ps_p, \
                 tc.tile_pool(name="a_pt", bufs=17) as pt_p, \
                 tc.tile_pool(name="a_sb", bufs=2) as at_sb_p, \
                 tc.tile_pool(name="a_r", bufs=8) as r_p, \
                 tc.tile_pool(name="a_out", bufs=3) as out_p:

                def attn_head(qb, h):
                    nkc = 4 * (qb + 1)
                    # scores + exp for the whole k band, kept in SBUF
                    ptus = []
                    for kc in range(nkc):
                        j = kc - 4 * qb  # >=0 on the diagonal band
                        st_ps = st_ps_p.tile([128, 512], F32, tag="st")
                        ptu = pt_p.tile([128, 512], BF16, tag="ptu")
                        if j < 0:
                            nc.tensor.matmul(
                                st_ps[:], kt_sb[:, kc, :],
                                qt_sb[:, qb * 4:(qb + 1) * 4, h, :],
                                start=True, stop=True)
                            nc.scalar.activation(
                                ptu, st_ps, mybir.ActivationFunctionType.Exp)
                        else:
                            # causal: q columns < 128j are fully masked
                            nc.tensor.matmul(
                                st_ps[:, 128 * j:512], kt_sb[:, kc, :],
                                qt_sb[:, qb * 4 + j:(qb + 1) * 4, h, :],
                                start=True, stop=True)
                            nc.scalar.activation(
                                ptu[:, 128 * j:512], st_ps[:, 128 * j:512],
                                mybir.ActivationFunctionType.Exp)
                            # zero the strictly-upper triangle of the
                            # diagonal block (keep where q - k >= 0)
                            nc.vector.tensor_mul(
                                ptu[:, 128 * j:128 * (j + 1)],
                                ptu[:, 128 * j:128 * (j + 1)], tri)
                        ptus.append(ptu)
                    # AV in two halves (2 PSUM accumulator banks); skip
                    # q sub-chunks that are fully masked on the diagonal
                    o_sb = at_sb_p.tile([128, 4, HD], BF16, tag="o_sb")
                    for hf in range(2):
                        o_ps = [o_ps_p.tile([128, HD + 1], F32, tag="o",
                                            name=f"ops{_t}") for _t in range(2)]
                        for kc in range(nkc):
                            for t2 in range(2):
                                t = hf * 2 + t2
                                if kc - 4 * qb > t:
                                    continue  # fully-masked block
                                nc.tensor.matmul(
                                    o_ps[t2][:],
                                    ptus[kc][:, t * 128:(t + 1) * 128],
                                    v_sb[:, kc, :],
                                    start=(kc == 0),
                                    stop=(kc == 4 * qb + t))
                        for t2 in range(2):
                            t = hf * 2 + t2
                            op = o_ps[t2][:]
                            r_t = r_p.tile([128, 1], F32, tag="r_t")
                            nc.vector.reciprocal(r_t, op[:, HD:HD + 1])
                            nc.vector.tensor_scalar_mul(o_sb[:, t, :],
                                                        op[:, 0:HD], r_t)
                    return o_sb

                def attn_head_fin(qb, h, o_sb):
                    # O^T via DMA-crossbar transposes (no PE/DVE cost); SP
                    # queue — ACT is exp-bound in this phase
                    for t in range(4):
                        nc.sync.dma_start_transpose(
                            out=ot_full[:, h, qb * 512 + t * 128:
                                        qb * 512 + (t + 1) * 128],
                            in_=o_sb[:, t, :])

                def outproj_row(r):
                    o_row = out_p.tile([128, 4, 512], BF16, tag="o_row")
                    for hs in range(4):
                        op_ps = op_ps_p.tile([128, 512], F32, tag="op")
                        for h in range(NHL):
                            nc.tensor.matmul(
                                op_ps[:],
                                ot_full[:, h, r * 128:(r + 1) * 128],
                                wc_sb[:, h, hs * 512:(hs + 1) * 512],
                                start=(h == 0), stop=(h == NHL - 1))
                        nc.vector.tensor_copy(o_row[:, hs, :], op_ps)
                    nc.sync.dma_start(
                        out=out_d[r * 128:(r + 1) * 128, :], in_=o_row)

                if phases == "12":
                    for qb in range(n_qb):
                        for h in range(NHL):
                            o_sb = attn_head(qb, h)
                            attn_head_fin(qb, h, o_sb)
                    nc.sync.dma_start(
                        out=out_d[0:128, :],
                        in_=ot_full.rearrange("p a b -> p (a b)")[:, 0:HID])
                else:
                    for qb in range(n_qb):
                        for h in range(NHL):
                            o_sb = attn_head(qb, h)
                            if qb >= 1:
                                outproj_row(4 * (qb - 1) + h)
                            attn_head_fin(qb, h, o_sb)
                    for r in range(4 * (n_qb - 1), 4 * n_qb):
                        outproj_row(r)
    nc.compile()
    return nc


def _rope_tables(positions_1d):
    half = HD // 2
    inv_freq = 1.0 / (THETA ** (np.arange(half, dtype=np.float64) / half))
    ang = positions_1d.astype(np.float64)[:, None] * inv_freq[None, :]
    return np.cos(ang), np.sin(ang)


def _to_f8(a, scale):
    import ml_dtypes
    return np.clip(a * scale, -240.0, 240.0).astype(ml_dtypes.float8_e4m3)


def _chunked(a):
    """[HID x N] -> [128, HID//128, N] with hid = c*128 + p."""
    hid, n = a.shape
    return np.ascontiguousarray(
        a.reshape(hid // 128, 128, n).transpose(1, 0, 2))


def _prep_shared(hidden_states, positions, Wq, Wk, Wv, Wc, q_scale, k_scale):
    """Per-batch and per-group host tensors, shared across cores."""
    import ml_dtypes
    bf16 = ml_dtypes.bfloat16
    c = float(HD) ** -0.25
    xq, xb = [], []
    for b in range(B):
        xt = np.ascontiguousarray(hidden_states[b].T)  # [HID, S]
        xq.append(_chunked(_to_f8(xt, S_X)))
        # s-block-major bf16: [128, n_sg, n_hc, 256]
        xb.append(np.ascontiguousarray(
            xt.astype(bf16).reshape(HID // 128, 128, S // 256, 256)
            .transpose(1, 2, 0, 3)))
    tabs = []
    for b in range(B):
        cos, sin = _rope_tables(np.asarray(positions[b]))
        tabs.append((cos.astype(bf16), sin.astype(bf16)))
    wq = [_chunked(_to_f8(Wq[:, g * DQ:(g + 1) * DQ], S_W)) for g in range(NKV)]
    wk = [_chunked(_to_f8(Wk[:, g * HD:(g + 1) * HD], S_W)) for g in range(NKV)]
    wv = [_chunked(Wv[:, g * HD:(g + 1) * HD].astype(bf16)) for g in range(NKV)]
    wc = [_chunked(Wc[g * DQ:(g + 1) * DQ, :].astype(bf16)) for g in range(NKV)]
    qs = np.tile(np.asarray(q_scale, np.float32) * c, NHL)
    # k comes out of the fp8 matmul S_QK-scaled, same as q, so its rmsnorm
    # stats share q's eps scaling and ks needs no extra factor
    ks = np.asarray(k_scale, np.float32) * c
    return xq, xb, tabs, wq, wk, wv, wc, qs, ks


def _core_inputs_all(inputs):
    xq, xb, tabs, wq, wk, wv, wc, qs, ks = _prep_shared(**inputs)
    in_maps = []
    for core in range(N_CORES):
        b, g = divmod(core, NKV)
        in_maps.append({
            "xq": xq[b], "xb": xb[b], "wq": wq[g], "wk": wk[g], "wv": wv[g],
            "wc": wc[g],
            "qs": qs, "ks": ks, "cos": tabs[b][0], "sin": tabs[b][1],
        })
    return in_maps


_CACHED = {}


def kernel(hidden_states, positions, Wq, Wk, Wv, Wc, q_scale, k_scale):
    from concourse import bass_utils

    inputs = dict(hidden_states=np.asarray(hidden_states, np.float32),
                  positions=np.asarray(positions),
                  Wq=np.asarray(Wq, np.float32), Wk=np.asarray(Wk, np.float32),
                  Wv=np.asarray(Wv, np.float32), Wc=np.asarray(Wc, np.float32),
                  q_scale=np.asarray(q_scale, np.float32),
                  k_scale=np.asarray(k_scale, np.float32))

    if "nc" not in _CACHED:
        _CACHED["nc"] = _build()
    nc = _CACHED["nc"]

    in_maps = _core_inputs_all(inputs)
    res = bass_utils.run_bass_kernel_spmd(nc, in_maps, core_ids=list(range(N_CORES)))
    out = np.zeros((B, S, HID), np.float32)
    for core in range(N_CORES):
        b, _ = divmod(core, NKV)
        out[b] += np.asarray(res.results[core]["out"], np.float32)
    return out


# revision 45
# speedup vs baseline: 1.1304x; 1.1304x over previous
"""Trainium2 Bass kernel for nn_BailingMoEAttention (B=2, S=2048, HID=2048,
NH=16, NKV=4, HD=128) on 8 NeuronCores.

Sharding: core c -> (batch b = c//4, kv-group g = c%4). Each core computes the
4 query heads sharing kv head g for batch b against its Wq/Wk/Wv column slices
and Wc row slice, producing a partial [S, HID] output; the host sums the 4
group partials per batch.

v2 design (per core):
  - Q projection runs as fp8e4 DoubleRow matmuls (HW-measured 2.06x over
    bf16): xt and Wq are host-scaled into fp8 range; rmsnorm cancels the
    scale (eps is pre-scaled to match). Q-side fp8 noise only perturbs
    softmax logits (~1e-2 relative), unlike the V/out-proj paths which
    propagate element-wise and would blow the 2e-2 budget - those stay bf16.
  - K/V projection in bf16 from an s-block-streamed bf16 x^T copy.
  - attention bf16: scores ST[k,q] = K-chunk^T @ Q^T, exp on ACT from PSUM;
    causal handling is exact at 128 granularity: fully-masked q sub-blocks
    are skipped in the scores matmul, exp, and AV accumulation; the diagonal
    [128,128] triangle is zeroed on Pool (affine_select) after exp.
  - softmax denominators ride as a ones-column in V; normalize is
    reciprocal+per-partition scale on DVE.
  - engine budget: ACT = exp + squares(+accum) + sqrt + qt drain; Pool =
    k-path (combk/kn/rope-k), rope-q heads 2-3, kt/v drains, diagonal
    affine_selects, half the out-proj drains; DVE = recip/comb/qn, rope-q
    heads 0-1, o_sb normalize, O^T copies, other half of out-proj drains.
  - all inputs arrive host-prearranged as [128, chunk, cols] so each tensor
    loads with a few large contiguous dma_starts (descriptor-efficient).

reps>1 compiles a NEFF running the whole body reps times back-to-back; used
by test.py to measure steady-state body time as the reps-delta.
"""
import os
import sys
sys.path.insert(0, "/opt/trn_rl_repo")

from contextlib import ExitStack

import numpy as np

import concourse.bass as bass
import concourse.tile as tile
from concourse import bacc, mybir
from concourse.masks import make_identity

F32 = mybir.dt.float32
BF16 = mybir.dt.bfloat16
F8 = mybir.dt.float8e4

B, S, HID = 2, 2048, 2048
NH, NKV, HD = 16, 4, 128
NHL = NH // NKV          # query heads per kv group (= per core)
DQ = NHL * HD
EPS = 1e-6
THETA = 10000.0
N_CORES = 8

S_X = 64.0               # fp8 scale on hidden_states^T (q path)
S_W = 32.0               # fp8 scale on Wq
S_QK = S_X * S_W         # q_ps comes out scaled by this; rmsnorm cancels it


def _build(reps=1, phases="123"):
    n_st = S // 128
    n_hc = HID // 128
    n_qb = S // 512
    n_sg = S // 256
    half = HD // 2
    DR = mybir.MatmulPerfMode.DoubleRow

    nc = bacc.Bacc("TRN2", target_bir_lowering=False, debug=False, num_devices=1)
    # q path: chunk-major fp8 x^T, resident
    xq_d = nc.dram_tensor("xq", [128, n_hc, S], F8, kind="ExternalInput").ap()
    # kv path: s-block-major bf16 x^T, streamed
    xb_d = nc.dram_tensor("xb", [128, n_sg, n_hc, 256], BF16,
                          kind="ExternalInput").ap()
    wq_d = nc.dram_tensor("wq", [128, n_hc, DQ], F8, kind="ExternalInput").ap()
    wk_d = nc.dram_tensor("wk", [128, n_hc, HD], F8, kind="ExternalInput").ap()
    wv_d = nc.dram_tensor("wv", [128, n_hc, HD], BF16,
                          kind="ExternalInput").ap()
    wc_d = nc.dram_tensor("wc", [128, NHL, HID], BF16, kind="ExternalInput").ap()
    qs_d = nc.dram_tensor("qs", [DQ], F32, kind="ExternalInput").ap()
    ks_d = nc.dram_tensor("ks", [HD], F32, kind="ExternalInput").ap()
    cos_d = nc.dram_tensor("cos", [S, half], BF16, kind="ExternalInput").ap()
    sin_d = nc.dram_tensor("sin", [S, half], BF16, kind="ExternalInput").ap()
    out_d = nc.dram_tensor("out", [S, HID], BF16, kind="ExternalOutput").ap()

    with tile.TileContext(nc) as tc, ExitStack() as ctx:
        const_p = ctx.enter_context(tc.tile_pool(name="const", bufs=1))
        big_p = ctx.enter_context(tc.tile_pool(name="big", bufs=1))

        ident = const_p.tile([128, 128], BF16)
        make_identity(nc, ident)
        # lower-triangle (q >= k) mask for diagonal score blocks
        tri = const_p.tile([128, 128], BF16)
        nc.gpsimd.memset(tri, 1.0)
        nc.gpsimd.affine_select(
            out=tri, in_=tri, compare_op=mybir.AluOpType.is_ge, fill=0.0,
            base=0, pattern=[[1, 128]], channel_multiplier=-1)
        eps_t = const_p.tile([128, 1], F32)
        # rmsnorm runs on S_QK-scaled q (and k stats are scale-matched via
        # the Square(scale=S_QK) trick), so eps scales by S_QK^2
        nc.vector.memset(eps_t, EPS * S_QK * S_QK)
        qs_b = const_p.tile([128, DQ], F32)
        nc.scalar.dma_start(out=qs_b, in_=bass.AP(tensor=qs_d.tensor, offset=0,
                                                  ap=[[0, 128]] + qs_d.ap))
        ks_b = const_p.tile([128, HD], F32)
        nc.scalar.dma_start(out=ks_b, in_=bass.AP(tensor=ks_d.tensor, offset=0,
                                                  ap=[[0, 128]] + ks_d.ap))
        cs_sb = const_p.tile([128, n_st, half], BF16)
        sn_sb = const_p.tile([128, n_st, half], BF16)
        nc.scalar.dma_start(out=cs_sb, in_=cos_d.rearrange("(t p) h -> p t h",
                                                           p=128))
        nc.scalar.dma_start(out=sn_sb, in_=sin_d.rearrange("(t p) h -> p t h",
                                                           p=128))

        xq_f = big_p.tile([128, n_hc, S], F8)
        wq_sb = big_p.tile([128, n_hc, DQ], F8)
        wk_sb = big_p.tile([128, n_hc, HD], F8)
        wv_sb = big_p.tile([128, n_hc, HD], BF16)
        wc_sb = big_p.tile([128, NHL, HID], BF16)
        qt_sb = big_p.tile([128, n_st, NHL, 128], BF16)  # [d,(st,head,qq)]
        kt_sb = big_p.tile([128, n_st, 128], BF16)       # [d,(chunk,kk)]
        v_sb = big_p.tile([128, n_st, HD + 1], BF16)     # [kk,(chunk, d|ones)]
        nc.vector.memset(v_sb[:, :, HD:HD + 1], 1.0)

        for _rep in range(reps):
            # weight/x loads: few large contiguous DMAs, ordered so the first
            # QKV matmuls unblock earliest
            # small first pieces so the first matmuls unblock within ~3us;
            # late pieces ride the otherwise-idle ACT queue, wc on Pool
            nc.sync.dma_start(out=wq_sb[:, 0:4, :], in_=wq_d[:, 0:4, :])
            nc.gpsimd.dma_start(out=xq_f[:, 0:4, :], in_=xq_d[:, 0:4, :])
            nc.sync.dma_start(out=wk_sb, in_=wk_d)
            nc.sync.dma_start(out=wv_sb[:, 0:8, :], in_=wv_d[:, 0:8, :])
            nc.sync.dma_start(out=wq_sb[:, 4:8, :], in_=wq_d[:, 4:8, :])
            nc.gpsimd.dma_start(out=xq_f[:, 4:16, :], in_=xq_d[:, 4:16, :])
            nc.scalar.dma_start(out=wq_sb[:, 8:16, :], in_=wq_d[:, 8:16, :])
            nc.scalar.dma_start(out=wv_sb[:, 8:16, :], in_=wv_d[:, 8:16, :])
            nc.gpsimd.dma_start(out=wc_sb[:, 0:2, :], in_=wc_d[:, 0:2, :])
            nc.gpsimd.dma_start(out=wc_sb[:, 2:4, :], in_=wc_d[:, 2:4, :])

            # ============ Phase 1: QKV + norm + rope + transposes ============
            with tc.tile_pool(name="p1xb", bufs=3) as xb_p, \
                 tc.tile_pool(name="p1q", bufs=3, space="PSUM") as qps_p, \
                 tc.tile_pool(name="p1kv", bufs=3, space="PSUM") as kvps_p, \
                 tc.tile_pool(name="p1t", bufs=2, space="PSUM") as tqk_p, \
                 tc.tile_pool(name="p1tmp", bufs=4) as tmp_p:
                def p1_stats(st, q_ps, kv_ps):
                    # rms stats: squares+accumulate on ACT, per head; k stats
                    # are computed on S_QK*k so one eps/bias serves both
                    junk = tmp_p.tile([128, 128], BF16, tag="junk")
                    ssq = tmp_p.tile([128, NHL + 1], F32, tag="ssq")
                    for h in range(NHL):
                        nc.scalar.activation(
                            junk, q_ps[:, h * HD:(h + 1) * HD],
                            mybir.ActivationFunctionType.Square,
                            accum_out=ssq[:, h:h + 1])
                    nc.scalar.activation(
                        junk, kv_ps[:, 0:HD],
                        mybir.ActivationFunctionType.Square,
                        accum_out=ssq[:, NHL:NHL + 1])
                    rstd = tmp_p.tile([128, NHL + 1], F32, tag="rstd")
                    nc.scalar.activation(rstd, ssq,
                                         mybir.ActivationFunctionType.Sqrt,
                                         bias=eps_t, scale=1.0 / HD)
                    nc.vector.reciprocal(rstd, rstd)
                    # q+k normalize into ONE tile so rope runs as 6 wide Pool
                    # ops over 5 "heads" (4 q heads + k) — GPSIMD ops carry a
                    # large unmodeled fixed cost on HW, so minimize their count
                    qkn = tmp_p.tile([128, NHL + 1, HD], BF16, tag="qkn")
                    comb = tmp_p.tile([128, DQ], F32, tag="comb")
                    rstd_b = bass.AP(
                        tensor=rstd.tensor, offset=rstd.offset,
                        ap=[rstd.ap[0], [rstd.ap[-1][0], NHL], [0, HD]])
                    nc.vector.tensor_mul(
                        comb.rearrange("p (h d) -> p h d", d=HD),
                        qs_b.rearrange("p (h d) -> p h d", d=HD), rstd_b)
                    nc.vector.tensor_mul(
                        qkn[:, 0:NHL, :], q_ps.rearrange("p (h d) -> p h d",
                                                         d=HD), comb.rearrange(
                            "p (h d) -> p h d", d=HD))
                    # k: combk = ks * rstd[k] (ks pre-scaled by S_QK on host)
                    combk = tmp_p.tile([128, HD], F32, tag="combk")
                    nc.gpsimd.tensor_scalar_mul(combk, ks_b,
                                                rstd[:, NHL:NHL + 1])
                    nc.vector.tensor_mul(qkn[:, NHL, :], kv_ps[:, 0:HD], combk)
                    nc.vector.tensor_copy(v_sb[:, st, 0:HD], kv_ps[:, HD:2 * HD])
                    return qkn

                def p1_finish(st, qkn):
                    # rope (bf16): 6 Pool ops over all 5 heads at once
                    qkr = tmp_p.tile([128, NHL + 1, HD], BF16, tag="qkr")
                    s3 = qkn.rearrange("p h (two d) -> p h two d", two=2)
                    d3 = qkr.rearrange("p h (two d) -> p h two d", two=2)
                    x1, x2 = s3[:, :, 0, :], s3[:, :, 1, :]
                    o1, o2 = d3[:, :, 0, :], d3[:, :, 1, :]
                    cs_t = cs_sb[:, st, :]
                    sn_t = sn_sb[:, st, :]
                    cb = bass.AP(tensor=cs_t.tensor, offset=cs_t.offset,
                                 ap=[cs_t.ap[0], [0, NHL + 1]] + cs_t.ap[1:])
                    sb = bass.AP(tensor=sn_t.tensor, offset=sn_t.offset,
                                 ap=[sn_t.ap[0], [0, NHL + 1]] + sn_t.ap[1:])
                    t1 = tmp_p.tile([128, NHL + 1, half], BF16, tag="ropet1")
                    t2 = tmp_p.tile([128, NHL + 1, half], BF16, tag="ropet2")
                    nc.gpsimd.tensor_mul(t1, x1, cb)
                    nc.gpsimd.tensor_mul(t2, x2, sb)
                    nc.gpsimd.tensor_sub(o1, t1, t2)
                    nc.gpsimd.tensor_mul(t1, x2, cb)
                    nc.gpsimd.tensor_mul(t2, x1, sb)
                    nc.gpsimd.tensor_add(o2, t1, t2)

                    # combined q+k transpose tile (fits 8 PSUM banks total)
                    tqk_ps = tqk_p.tile([128, DQ + HD], BF16, tag="tqk")
                    for h in range(NHL + 1):
                        nc.tensor.transpose(tqk_ps[:, h * HD:(h + 1) * HD],
                                            qkr[:, h, :], ident)
                    nc.scalar.activation(qt_sb[:, st, :, :], tqk_ps[:, 0:DQ],
                                         mybir.ActivationFunctionType.Copy)
                    nc.vector.tensor_copy(kt_sb[:, st, :], tqk_ps[:, DQ:DQ + HD])

                # software pipeline, 2 tail stages deep: for iteration st we
                # emit MMs(st), stats(st-1), finish(st-2) so every engine's
                # strict FIFO always holds ready work (a single monolithic
                # tail head-of-line-blocks each queue on the slow chain)
                stats_q, fin_q = [], []
                for sg in range(n_sg):
                    xb_t = xb_p.tile([128, n_hc, 256], BF16, tag="xb")
                    nc.sync.dma_start(out=xb_t, in_=xb_d[:, sg, :, :])
                    for t in range(2):
                        st = sg * 2 + t
                        q_ps = qps_p.tile([128, DQ], F32, tag="qps")
                        kv_ps = kvps_p.tile([128, 2 * HD], F32, tag="kvps")
                        for c8 in range(n_hc // 2):
                            lhsT = xq_f[:, 2 * c8:2 * c8 + 2,
                                        st * 128:(st + 1) * 128]
                            nc.tensor.matmul(
                                q_ps[:], lhsT, wq_sb[:, 2 * c8:2 * c8 + 2, :],
                                start=(c8 == 0), stop=(c8 == n_hc // 2 - 1),
                                perf_mode=DR)
                            nc.tensor.matmul(
                                kv_ps[:, 0:HD], lhsT,
                                wk_sb[:, 2 * c8:2 * c8 + 2, :],
                                start=(c8 == 0), stop=(c8 == n_hc // 2 - 1),
                                perf_mode=DR)
                        for c in range(n_hc):
                            nc.tensor.matmul(
                                kv_ps[:, HD:2 * HD],
                                xb_t[:, c, t * 128:(t + 1) * 128],
                                wv_sb[:, c, :],
                                start=(c == 0), stop=(c == n_hc - 1))
                        stats_q.append((st, q_ps, kv_ps))
                        if len(stats_q) > 1:
                            s_st, s_q, s_kv = stats_q.pop(0)
                            fin_q.append((s_st, p1_stats(s_st, s_q, s_kv)))
                        if len(fin_q) > 1:
                            p1_finish(*fin_q.pop(0))
                for s_st, s_q, s_kv in stats_q:
                    fin_q.append((s_st, p1_stats(s_st, s_q, s_kv)))
                for args in fin_q:
                    p1_finish(*args)

            if phases == "1":
                nc.sync.dma_start(
                    out=out_d[0:128, :],
                    in_=qt_sb.rearrange("p a b c -> p (a b c)")[:, 0:HID])
                continue
            # full normalized O^T, [d, (head, q)] bf16 (2 MB)
            ot_full = big_p.tile([128, NHL, S], BF16)

            # ===== Phase 2+3: attention with interleaved out-proj rows =====
            with tc.tile_pool(name="a_st", bufs=2, space="PSUM") as st_ps_p, \
                 tc.tile_pool(name="a_o", bufs=2, space="PSUM") as o_ps_p, \
                 tc.tile_pool(name="a_ot", bufs=1, space="PSUM") as ot_ps_p, \
                 tc.tile_pool(name="a_op", bufs=3, space="PSUM") as op_ps_p, \
                 tc.tile_pool(name="a_pt", bufs=17) as pt_p, \
                 tc.tile_pool(name="a_sb", bufs=2) as at_sb_p, \
                 tc.tile_pool(name="a_r", bufs=8) as r_p, \
                 tc.tile_pool(name="a_out", bufs=3) as out_p:

                def attn_head(qb, h):
                    nkc = 4 * (qb + 1)
                    # scores + exp for the whole k band, kept in SBUF
                    ptus = []
                    for kc in range(nkc):
                        j = kc - 4 * qb  # >=0 on the diagonal band
                        st_ps = st_ps_p.tile([128, 512], F32, tag="st")
                        ptu = pt_p.tile([128, 512], BF16, tag="ptu")
                        if j < 0:
                            nc.tensor.matmul(
                                st_ps[:], kt_sb[:, kc, :],
                                qt_sb[:, qb * 4:(qb + 1) * 4, h, :],
                                start=True, stop=True)
                            nc.scalar.activation(
                                ptu, st_ps, mybir.ActivationFunctionType.Exp)
                        else:
                            # causal: q columns < 128j are fully masked
                            nc.tensor.matmul(
                                st_ps[:, 128 * j:512], kt_sb[:, kc, :],
                                qt_sb[:, qb * 4 + j:(qb + 1) * 4, h, :],
                                start=True, stop=True)
                            nc.scalar.activation(
                                ptu[:, 128 * j:512], st_ps[:, 128 * j:512],
                                mybir.ActivationFunctionType.Exp)
                            # zero the strictly-upper triangle of the
                            # diagonal block (keep where q - k >= 0)
                            nc.vector.tensor_mul(
                                ptu[:, 128 * j:128 * (j + 1)],
                                ptu[:, 128 * j:128 * (j + 1)], tri)
                        ptus.append(ptu)
                    # AV in two halves (2 PSUM accumulator banks); skip
                    # q sub-chunks that are fully masked on the diagonal
                    o_sb = at_sb_p.tile([128, 4, HD], BF16, tag="o_sb")
                    for hf in range(2):
                        o_ps = [o_ps_p.tile([128, HD + 1], F32, tag="o",
                                            name=f"ops{_t}") for _t in range(2)]
                        for kc in range(nkc):
                            for t2 in range(2):
                                t = hf * 2 + t2
                                if kc - 4 * qb > t:
                                    continue  # fully-masked block
                                nc.tensor.matmul(
                                    o_ps[t2][:],
                                    ptus[kc][:, t * 128:(t + 1) * 128],
                                    v_sb[:, kc, :],
                                    start=(kc == 0),
                                    stop=(kc == 4 * qb + t))
                        for t2 in range(2):
                            t = hf * 2 + t2
                            op = o_ps[t2][:]
                            r_t = r_p.tile([128, 1], F32, tag="r_t")
                            nc.vector.reciprocal(r_t, op[:, HD:HD + 1])
                            nc.vector.tensor_scalar_mul(o_sb[:, t, :],
                                                        op[:, 0:HD], r_t)
                    return o_sb

                def attn_head_fin(qb, h, o_sb):
                    # deferred past the interleaved out-proj so the PE isn't
                    # FIFO-blocked waiting on the DVE normalize
                    ot_ps = ot_ps_p.tile([128, 512], BF16, tag="ot")
                    for t in range(4):
                        nc.tensor.transpose(ot_ps[:, t * 128:(t + 1) * 128],
                                            o_sb[:, t, :], ident)
                    nc.vector.tensor_copy(ot_full[:, h, qb * 512:(qb + 1) * 512],
                                          ot_ps)

                def outproj_row(r):
                    o_row = out_p.tile([128, 4, 512], BF16, tag="o_row")
                    for hs in range(4):
                        op_ps = op_ps_p.tile([128, 512], F32, tag="op")
                        for h in range(NHL):
                            nc.tensor.matmul(
                                op_ps[:],
                                ot_full[:, h, r * 128:(r + 1) * 128],
                                wc_sb[:, h, hs * 512:(hs + 1) * 512],
                                start=(h == 0), stop=(h == NHL - 1))
                        nc.vector.tensor_copy(o_row[:, hs, :], op_ps)
                    nc.sync.dma_start(
                        out=out_d[r * 128:(r + 1) * 128, :], in_=o_row)

                if phases == "12":
                    for qb in range(n_qb):
                        for h in range(NHL):
                            o_sb = attn_head(qb, h)
                            attn_head_fin(qb, h, o_sb)
                    nc.sync.dma_start(
                        out=out_d[0:128, :],
                        in_=ot_full.rearrange("p a b -> p (a b)")[:, 0:HID])
                else:
                    for qb in range(n_qb):
                        for h in range(NHL):
                            o_sb = attn_head(qb, h)
                            if qb >= 1:
                                outproj_row(4 * (qb - 1) + h)
                            attn_head_fin(qb, h, o_sb)
                    for r in range(4 * (n_qb - 1), 4 * n_qb):
                        outproj_row(r)
    nc.compile()
    return nc


def _rope_tables(positions_1d):
    half = HD // 2
    inv_freq = 1.0 / (THETA ** (np.arange(half, dtype=np.float64) / half))
    ang = positions_1d.astype(np.float64)[:, None] * inv_freq[None, :]
    return np.cos(ang), np.sin(ang)


def _to_f8(a, scale):
    import ml_dtypes
    return np.clip(a * scale, -240.0, 240.0).astype(ml_dtypes.float8_e4m3)


def _chunked(a):
    """[HID x N] -> [128, HID//128, N] with hid = c*128 + p."""
    hid, n = a.shape
    return np.ascontiguousarray(
        a.reshape(hid // 128, 128, n).transpose(1, 0, 2))


def _prep_shared(hidden_states, positions, Wq, Wk, Wv, Wc, q_scale, k_scale):
    """Per-batch and per-group host tensors, shared across cores."""
    import ml_dtypes
    bf16 = ml_dtypes.bfloat16
    c = float(HD) ** -0.25
    xq, xb = [], []
    for b in range(B):
        xt = np.ascontiguousarray(hidden_states[b].T)  # [HID, S]
        xq.append(_chunked(_to_f8(xt, S_X)))
        # s-block-major bf16: [128, n_sg, n_hc, 256]
        xb.append(np.ascontiguousarray(
            xt.astype(bf16).reshape(HID // 128, 128, S // 256, 256)
            .transpose(1, 2, 0, 3)))
    tabs = []
    for b in range(B):
        cos, sin = _rope_tables(np.asarray(positions[b]))
        tabs.append((cos.astype(bf16), sin.astype(bf16)))
    wq = [_chunked(_to_f8(Wq[:, g * DQ:(g + 1) * DQ], S_W)) for g in range(NKV)]
    wk = [_chunked(_to_f8(Wk[:, g * HD:(g + 1) * HD], S_W)) for g in range(NKV)]
    wv = [_chunked(Wv[:, g * HD:(g + 1) * HD].astype(bf16)) for g in range(NKV)]
    wc = [_chunked(Wc[g * DQ:(g + 1) * DQ, :].astype(bf16)) for g in range(NKV)]
    qs = np.tile(np.asarray(q_scale, np.float32) * c, NHL)
    # k comes out of the fp8 matmul S_QK-scaled, same as q, so its rmsnorm
    # stats share q's eps scaling and ks needs no extra factor
    ks = np.asarray(k_scale, np.float32) * c
    return xq, xb, tabs, wq, wk, wv, wc, qs, ks


def _core_inputs_all(inputs):
    xq, xb, tabs, wq, wk, wv, wc, qs, ks = _prep_shared(**inputs)
    in_maps = []
    for core in range(N_CORES):
        b, g = divmod(core, NKV)
        in_maps.append({
            "xq": xq[b], "xb": xb[b], "wq": wq[g], "wk": wk[g], "wv": wv[g],
            "wc": wc[g],
            "qs": qs, "ks": ks, "cos": tabs[b][0], "sin": tabs[b][1],
        })
    return in_maps


_CACHED = {}


def kernel(hidden_states, positions, Wq, Wk, Wv, Wc, q_scale, k_scale):
    from concourse import bass_utils

    inputs = dict(hidden_states=np.asarray(hidden_states, np.float32),
                  positions=np.asarray(positions),
                  Wq=np.asarray(Wq, np.float32), Wk=np.asarray(Wk, np.float32),
                  Wv=np.asarray(Wv, np.float32), Wc=np.asarray(Wc, np.float32),
                  q_scale=np.asarray(q_scale, np.float32),
                  k_scale=np.asarray(k_scale, np.float32))

    if "nc" not in _CACHED:
        _CACHED["nc"] = _build()
    nc = _CACHED["nc"]

    in_maps = _core_inputs_all(inputs)
    res = bass_utils.run_bass_kernel_spmd(nc, in_maps, core_ids=list(range(N_CORES)))
    out = np.zeros((B, S, HID), np.float32)
    for core in range(N_CORES):
        b, _ = divmod(core, NKV)
        out[b] += np.asarray(res.results[core]["out"], np.float32)
    return out
